# revision 1
# baseline (speedup 1.0000x reference)
"""Trainium2 8-core kernel for nn_AlignedGloveLayer (retrieval 1-NN mismatch loss).

Problem: a = mapped[indexes] ([4096, 256]); d2[k, j] = |a_k - target_j|^2 over
30000 targets; loss = mean over k of (argmin_j d2[k, j] != indexes[k]).

Only the comparison min_j d2 vs d2[:, indexes[k]] matters (sqrt is monotone and
the a2 term is constant per row), so the device computes, per query,
m_k = min_j (b2_j - 2 a_k . t_j). The mismatch decision and the final mean are
assembled on the host, with an exact fp64 fallback for any query whose margin
is within the device-arithmetic error bound (fp8 matmul + fp16 min).

Sharding (2x4 grid): cores 0-3 take 1024 queries each over the first half of
the targets; cores 4-7 take the same query slices over the second half; the
host mins the two halves. Per core, flipped orientation:
  psum[t, q] = sum_d T[t, d] * (-2 a[q, d])   (targets on psum partitions)
  ScalarE:  val16[t, q] = psum + (b2[t] - SHIFT)   (per-partition bias; a few
            chunks go through VectorE tensor_scalar instead to balance load)
  VectorE:  acc16 = min(acc16, val16)              (fp16, 2x mode, 4 rotating accs)
  final:    DMA the 4 fp16 accumulators out; host does the cross-partition min
Targets are padded 30000 -> 30720 = 240*128; padded rows get b2' = 60000.
"""
import os
import sys

for _p in ("/opt/trn_rl_repo", "/root/.axon_site/_ro/trn_rl_repo"):
    if os.path.isdir(_p) and _p not in sys.path:
        sys.path.append(_p)

from contextlib import ExitStack

import ml_dtypes
import numpy as np

NX, NY, D, K = 30000, 30000, 256, 4096
NCORES = 8
P = 128
DC = D // P          # 2 contraction chunks
NQ = 1024            # queries per core (cores c and c+4 share a query slice)
QB = NQ // P         # 8 query blocks
TCH = 240            # total target chunks: 240*128 = 30720 >= 30000
TCHH = TCH // 2      # target chunks per core (half of the targets)
NYP = TCH * P
GROUP = 4            # target chunks per DMA batch
DVE_AFFINE_EVERY = 6 # every Nth chunk's affine goes to VectorE instead of ScalarE
SHIFT = 512.0        # centers val16 in fp16 range
INIT = 60000.0       # min-accumulator init (> any real val')
PADVAL = 60000.0     # padded targets' shifted b2 (never the min)
DELTA = 18.0         # device error bound for host fallback flagging (fp8 matmul)

_CACHE: dict = {}


def _build_nc():
    import concourse.tile as tile
    from concourse import bacc, mybir
    nc = bacc.Bacc("TRN2", target_bir_lowering=False)
    at_d = nc.dram_tensor("at", [P, DC, NQ], mybir.dt.float8e4, kind="ExternalInput")
    tt_d = nc.dram_tensor("tt", [P, TCHH, DC, P], mybir.dt.float8e4, kind="ExternalInput")
    b2_d = nc.dram_tensor("b2c", [P, TCHH], mybir.dt.float32, kind="ExternalInput")
    b2hl_d = nc.dram_tensor("b2hl", [2, TCHH, P], mybir.dt.bfloat16, kind="ExternalInput")
    m_d = nc.dram_tensor("m", [P, 4, NQ], mybir.dt.float16, kind="ExternalOutput")
    m32_d = nc.dram_tensor("m32", [P, 2, NQ], mybir.dt.float32, kind="ExternalOutput")

    with tile.TileContext(nc) as tc:
        with ExitStack() as ctx:
            sb = ctx.enter_context(tc.tile_pool(name="sb", bufs=1))
            stream = ctx.enter_context(tc.tile_pool(name="stream", bufs=8))
            vals = ctx.enter_context(tc.tile_pool(name="vals", bufs=6))
            psum = ctx.enter_context(tc.tile_pool(name="psum", bufs=4, space="PSUM"))

            at = sb.tile([P, DC, NQ], mybir.dt.float8e4)
            nc.scalar.dma_start(at[:], at_d[:])
            b2c = sb.tile([P, TCHH], mybir.dt.float32)
            nc.scalar.dma_start(b2c[:], b2_d[:])
            b2hl = sb.tile([2, TCHH, P], mybir.dt.bfloat16)
            nc.scalar.dma_start(b2hl[:], b2hl_d[:])
            ones2 = sb.tile([2, 512], mybir.dt.bfloat16)
            nc.vector.memset(ones2[:], 1.0)
            NACC = 4     # rotating fp16 min accumulators (ScalarE-route chunks)
            accs = []
            for i in range(NACC):
                a_t = sb.tile([P, NQ], mybir.dt.float16, tag=f"acc{i}", name=f"acc{i}")
                nc.gpsimd.memset(a_t[:], INIT)
                accs.append(a_t)
            NACC32 = 2   # fp32 accumulators for the direct min-on-psum route
            accs32 = []
            for i in range(NACC32):
                a32 = sb.tile([P, NQ], mybir.dt.float32, tag=f"acc32_{i}", name=f"acc32_{i}")
                nc.gpsimd.memset(a32[:], INIT)
                accs32.append(a32)

            for g in range(TCHH // GROUP):
                tt = stream.tile([P, GROUP, DC, P], mybir.dt.float8e4, tag="tt")
                nc.sync.dma_start(tt[:], tt_d[:, g * GROUP:(g + 1) * GROUP])
                for j in range(GROUP):
                    t = g * GROUP + j
                    direct = (t % 11 in (3, 7, 10))
                    ps = psum.tile([P, NQ], mybir.dt.float32)
                    # fp8 DoubleRow: full 256-deep contraction in one matmul,
                    # N<=512 psum limit -> two query halves
                    for h in range(NQ // 512):
                        nc.tensor.matmul(
                            ps[:, h * 512:(h + 1) * 512],
                            tt[:, j, :, :],
                            at[:, :, h * 512:(h + 1) * 512],
                            start=True, stop=not direct,
                            perf_mode=mybir.MatmulPerfMode.DoubleRow,
                        )
                    if direct:
                        # fold b2' (split into bf16 hi+lo rows) into psum on PE,
                        # then VectorE min-accumulates straight from psum
                        for h in range(NQ // 512):
                            nc.tensor.matmul(
                                ps[:, h * 512:(h + 1) * 512],
                                b2hl[:, t, :], ones2[:],
                                start=False, stop=True,
                            )
                        a32 = accs32[(t // 5) % NACC32]
                        nc.vector.tensor_tensor(a32[:], a32[:], ps[:], mybir.AluOpType.min)
                    else:
                        # ScalarE route: val = psum + b2' (per-partition bias)
                        val = vals.tile([P, NQ], mybir.dt.float16, tag="val")
                        nc.scalar.activation(
                            val[:], ps[:], mybir.ActivationFunctionType.Identity,
                            bias=b2c[:, t:t + 1], scale=1.0,
                        )
                        a_t = accs[t % NACC]
                        nc.vector.tensor_tensor(a_t[:], a_t[:], val[:], mybir.AluOpType.min)

            for i in range(NACC):
                nc.sync.dma_start(m_d[:, i], accs[i][:])
            for i in range(NACC32):
                nc.sync.dma_start(m32_d[:, i], accs32[i][:])

    nc.compile()
    return nc


def _get_nc():
    if "nc" not in _CACHE:
        _CACHE["nc"] = _build_nc()
    return _CACHE["nc"]


def kernel(mapped: np.ndarray, target: np.ndarray, indexes: np.ndarray) -> np.ndarray:
    from concourse.bass_utils import run_bass_kernel_spmd

    mapped = np.asarray(mapped, dtype=np.float32)
    target = np.asarray(target, dtype=np.float32)
    idx = np.asarray(indexes).astype(np.int64)

    # ---- host-side sharding / marshalling ----
    a = mapped[idx]                                   # [K, D]
    at_all = np.ascontiguousarray((-2.0 * a).T)       # [D, K]

    b2_64 = (target.astype(np.float64) ** 2).sum(1)   # exact fp64 row norms
    b2p = np.full(NYP, PADVAL + SHIFT, dtype=np.float64)
    b2p[:NY] = b2_64
    b2c_all = (b2p - SHIFT).astype(np.float32).reshape(TCH, P).T  # [P, TCH]

    tpad = np.zeros((NYP, D), dtype=np.float32)
    tpad[:NY] = target
    # tt[d_low, tc, dc, t] = target[tc*128 + t, dc*128 + d_low]
    tt_all = np.ascontiguousarray(
        tpad.reshape(TCH, P, DC, P).transpose(3, 0, 2, 1)
    ).astype(ml_dtypes.float8_e4m3)                    # [P, TCH, DC, P] fp8e4m3

    tt_half = [np.ascontiguousarray(tt_all[:, :TCHH]),
               np.ascontiguousarray(tt_all[:, TCHH:])]
    b2_half = [np.ascontiguousarray(b2c_all[:, :TCHH]),
               np.ascontiguousarray(b2c_all[:, TCHH:])]
    b2s = (b2p - SHIFT).astype(np.float32)             # [NYP] shifted b2
    b2h_rows = b2s.astype(ml_dtypes.bfloat16)
    b2l_rows = (b2s - b2h_rows.astype(np.float32)).astype(ml_dtypes.bfloat16)
    b2hl_all = np.stack([b2h_rows, b2l_rows]).reshape(2, TCH, P)  # [2, TCH, P]
    b2hl_half = [np.ascontiguousarray(b2hl_all[:, :TCHH]),
                 np.ascontiguousarray(b2hl_all[:, TCHH:])]

    at_cores = []
    for cq in range(K // NQ):                          # 4 query slices
        at_cores.append(np.ascontiguousarray(
            at_all[:, cq * NQ:(cq + 1) * NQ].reshape(DC, P, NQ).transpose(1, 0, 2)
        ).astype(ml_dtypes.float8_e4m3))               # [P, DC, NQ] fp8e4m3

    in_maps = []
    for c in range(NCORES):
        half = c // 4
        in_maps.append({"at": at_cores[c % 4], "tt": tt_half[half],
                        "b2c": b2_half[half], "b2hl": b2hl_half[half]})

    # ---- run on the 8 NeuronCores (host numpy fallback if the device path
    # fails repeatedly — correctness insurance, ~30s instead of ~120us) ----
    m_dev = None
    last_exc = None
    for attempt in range(3):
        try:
            nc = _get_nc()
            kwargs = {}
            if os.environ.get("KERNEL_TRACE_DIR"):
                kwargs["tmpdir"] = os.environ["KERNEL_TRACE_DIR"]
            res = run_bass_kernel_spmd(
                nc, in_maps, core_ids=list(range(NCORES)), **kwargs
            )
            _CACHE["last_res"] = res  # exec_time_ns/profile when BASS_TRACE=1
            # m[p, i, q] on core c: acc i, target-partition p, query
            # (c%4)*1024 + q; min over p and i here, then across the halves
            m_halves = [np.minimum(res.results[c]["m"].min(axis=(0, 1)),
                                   res.results[c]["m32"].min(axis=(0, 1)))
                        for c in range(NCORES)]
            m_dev = np.minimum(
                np.concatenate(m_halves[:4]), np.concatenate(m_halves[4:])
            ).astype(np.float64)                       # [K] shifted mins
            break
        except Exception as e:  # noqa: BLE001 - retry/fallback on any device error
            last_exc = e
            _CACHE.pop("nc", None)
    if m_dev is None:
        sys.stderr.write(f"kernel: device path failed ({last_exc}); host fallback\n")
        m_dev = np.empty(K, dtype=np.float64)
        tT = target.T.astype(np.float32)
        for i in range(0, K, 256):
            s = a[i:i + 256] @ tT
            m_dev[i:i + 256] = (
                b2_64[None, :NY].astype(np.float32) - 2.0 * s
            ).min(1).astype(np.float64) - SHIFT

    # ---- host decision + exact fallback ----
    t64 = None
    v = b2_64[idx] - 2.0 * np.einsum(
        "kd,kd->k", a.astype(np.float64), target[idx].astype(np.float64)
    ) - SHIFT                                          # shifted val at own index

    mismatch = m_dev < v - DELTA                       # confidently mismatched
    flagged = np.nonzero(~mismatch)[0]
    for i in range(0, len(flagged), 64):
        blk = flagged[i:i + 64]
        if t64 is None:
            t64 = target.astype(np.float64)
        d2 = b2_64[None, :] - 2.0 * (a[blk].astype(np.float64) @ t64.T)
        mismatch[blk] = np.argmin(d2, axis=1) != idx[blk]

    return np.asarray(mismatch.mean(), dtype=np.float32)


if __name__ == "__main__":
    rng = np.random.default_rng(1)
    mapped = rng.standard_normal((NX, D)).astype(np.float32)
    target = rng.standard_normal((NY, D)).astype(np.float32)
    indexes = rng.integers(0, NY, size=K).astype(np.int32)
    out = kernel(mapped=mapped, target=target, indexes=indexes)
    print("kernel output:", out, out.shape, out.dtype)



# revision 2
# speedup vs baseline: 4.2111x; 4.2111x over previous
"""Trainium2 8-core kernel for nn_AlignedGloveLayer (retrieval 1-NN mismatch loss).

Problem: a = mapped[indexes] ([4096, 256]); d2[k, j] = |a_k - target_j|^2 over
30000 targets; loss = mean over k of (argmin_j d2[k, j] != indexes[k]).

Strategy (witness counting): query k is mismatched iff SOME target j has
d2[k, j] < d2[k, indexes[k]]. The device searches a fixed sampled subset of
S targets for witnesses with margin DELTA (covering fp8/bf16 device error):
any witness found proves mismatch; queries with no witness are resolved
exactly on the host (expected only a handful for random data, since a query's
own-index distance typically ranks ~uniformly among 30000 distances).

Device layout (queries on PSUM partitions, targets on the free dim):
  2x4 grid: cores 0-3 take 1024 queries each over the first S/2 sampled
  targets; cores 4-7 the same query slices over the second S/2.
  Per core, 8 query blocks of 128; per block one PSUM tile [128, S_c]:
    psum[q, t] = sum_d T[t, d] * (-2 a[q, d])   (fp8 DoubleRow, 256-deep)
  then ONE fused instruction per tile produces a per-query witness measure:
    ACT route: psum += b2' via a bf16 fold matmul, then
       out = Relu((v - C - DELTA) - psum), accum_out[q] = sum(out)  (>0 iff witness)
    DVE route: scalar_tensor_tensor
       out = (psum - (v - DELTA)) is_lt (-b2_rep), accum_out[q] = count
  Only the [128, n_tiles] accum table is DMA'd out (4KB/core).
"""
import os
import sys

for _p in ("/opt/trn_rl_repo", "/root/.axon_site/_ro/trn_rl_repo"):
    if os.path.isdir(_p) and _p not in sys.path:
        sys.path.append(_p)

from contextlib import ExitStack

import ml_dtypes
import numpy as np

NX, NY, D, K = 30000, 30000, 256, 4096
NCORES = 8
P = 128
DC = D // P          # 2 contraction k-tiles (fp8 DoubleRow: 256-deep)
NQ = 1024            # queries per core (cores c and c+4 share a query slice)
QB = NQ // P         # 8 query blocks
S_TOTAL = 2048       # sampled targets (device witness search set)
S_C = S_TOTAL // 2   # sampled targets per core (two halves)
SAMPLE_SEED = 12345
CENTER = 256.0       # b2 centering for the bf16 fold row
DELTA = 18.5         # witness margin >= device arithmetic error bound
ACT_SET = (2, 5)     # query blocks routed through ScalarE (with b2 fold)

_CACHE: dict = {}


def _build_nc():
    import concourse.tile as tile
    from concourse import bacc, mybir
    nc = bacc.Bacc("TRN2", target_bir_lowering=False)
    at_d = nc.dram_tensor("at", [P, DC, NQ], mybir.dt.float8e4, kind="ExternalInput")
    tt_d = nc.dram_tensor("tt", [P, DC, S_C], mybir.dt.float8e4, kind="ExternalInput")
    b2r_d = nc.dram_tensor("b2r", [1, S_C], mybir.dt.bfloat16, kind="ExternalInput")
    nb_d = nc.dram_tensor("nb", [P, S_C], mybir.dt.bfloat16, kind="ExternalInput")
    vba_d = nc.dram_tensor("vba", [P, QB], mybir.dt.float32, kind="ExternalInput")
    vbd_d = nc.dram_tensor("vbd", [P, QB], mybir.dt.float32, kind="ExternalInput")
    accw_d = nc.dram_tensor("accw", [P, QB], mybir.dt.float32, kind="ExternalOutput")

    NDT = S_C // 1024  # psum tiles per query block (1024-target dtiles)
    assert NDT * 1024 == S_C and NDT == 1, "layout assumes one dtile per qb"

    with tile.TileContext(nc) as tc:
        with ExitStack() as ctx:
            sb = ctx.enter_context(tc.tile_pool(name="sb", bufs=1))
            dump = ctx.enter_context(tc.tile_pool(name="dump", bufs=3))
            psum = ctx.enter_context(tc.tile_pool(name="psum", bufs=4, space="PSUM"))

            at = sb.tile([P, DC, NQ], mybir.dt.float8e4)
            nc.scalar.dma_start(at[:], at_d[:])
            tt = sb.tile([P, DC, S_C], mybir.dt.float8e4)
            nc.sync.dma_start(tt[:], tt_d[:])
            b2r = sb.tile([1, S_C], mybir.dt.bfloat16)
            nc.scalar.dma_start(b2r[:], b2r_d[:])
            nb = sb.tile([P, S_C], mybir.dt.bfloat16)
            nc.sync.dma_start(nb[:], nb_d[:])
            vba = sb.tile([P, QB], mybir.dt.float32)
            nc.scalar.dma_start(vba[:], vba_d[:])
            vbd = sb.tile([P, QB], mybir.dt.float32)
            nc.scalar.dma_start(vbd[:], vbd_d[:])
            ones = sb.tile([1, P], mybir.dt.bfloat16)
            nc.gpsimd.memset(ones[:], 1.0)
            accw = sb.tile([P, QB], mybir.dt.float32)
            nc.gpsimd.memset(accw[:], 0.0)

            # Pull the ACT table load off the critical path: a tiny Relu
            # while the input DMAs are in flight.
            warm = sb.tile([P, 1], mybir.dt.float32)
            nc.gpsimd.memset(warm[:], 0.0)
            nc.scalar.activation(
                warm[:], warm[:], mybir.ActivationFunctionType.Relu,
                bias=0.0, scale=1.0,
            )

            for qb in range(QB):
                act_route = qb in ACT_SET
                ps = psum.tile([P, S_C], mybir.dt.float32)
                for h in range(S_C // 512):
                    t0 = h * 512
                    if act_route:
                        # fold centered b2 (bf16 row) into psum first
                        nc.tensor.matmul(
                            ps[:, t0:t0 + 512], ones[:], b2r[:, t0:t0 + 512],
                            start=True, stop=False,
                        )
                    nc.tensor.matmul(
                        ps[:, t0:t0 + 512],
                        at[:, :, qb * P:(qb + 1) * P],
                        tt[:, :, t0:t0 + 512],
                        start=not act_route, stop=True,
                        perf_mode=mybir.MatmulPerfMode.DoubleRow,
                    )
                vo = dump.tile([P, S_C], mybir.dt.float16, tag="vo")
                if act_route:
                    # accum[q] = sum_t relu((v - C - DELTA) - psum) > 0 iff witness
                    nc.scalar.activation(
                        vo[:], ps[:], mybir.ActivationFunctionType.Relu,
                        bias=vba[:, qb:qb + 1], scale=-1.0,
                        accum_out=accw[:, qb:qb + 1],
                    )
                else:
                    # accum[q] = #targets with psum - (v - DELTA) < -b2
                    nc.vector.scalar_tensor_tensor(
                        vo[:], ps[:], vbd[:, qb:qb + 1], nb[:],
                        op0=mybir.AluOpType.subtract,
                        op1=mybir.AluOpType.is_lt,
                        accum_out=accw[:, qb:qb + 1],
                    )

            nc.sync.dma_start(accw_d[:], accw[:])

    nc.compile()
    return nc


def _get_nc():
    if "nc" not in _CACHE:
        _CACHE["nc"] = _build_nc()
    return _CACHE["nc"]


def _marshal(mapped, target, idx):
    """Host-side sharding/quantization. Returns (in_maps, a, b2_64)."""
    a = mapped[idx]                                   # [K, D] fp32
    at_all = np.ascontiguousarray((-2.0 * a).T)       # [D, K]

    rng = np.random.default_rng(SAMPLE_SEED)
    sidx = np.sort(rng.permutation(NY)[:S_TOTAL])
    _CACHE["sidx"] = sidx
    tsub = target[sidx]                               # [S, D]

    b2_64 = (target.astype(np.float64) ** 2).sum(1)   # exact fp64 row norms
    b2s = b2_64[sidx]                                 # [S]

    # tt[p, dc, t] = tsub[t, dc*128 + p] in fp8
    tt_all = np.ascontiguousarray(
        tsub.reshape(S_TOTAL, DC, P).transpose(2, 1, 0)
    ).astype(ml_dtypes.float8_e4m3)                   # [P, DC, S]
    tt_half = [np.ascontiguousarray(tt_all[:, :, :S_C]),
               np.ascontiguousarray(tt_all[:, :, S_C:])]

    b2r_all = (b2s - CENTER).astype(ml_dtypes.bfloat16).reshape(1, S_TOTAL)
    b2r_half = [np.ascontiguousarray(b2r_all[:, :S_C]),
                np.ascontiguousarray(b2r_all[:, S_C:])]
    nb_all = (-b2s).astype(ml_dtypes.bfloat16)
    nb_rep = np.ascontiguousarray(
        np.broadcast_to(nb_all[None, :], (P, S_TOTAL))
    )
    nb_half = [np.ascontiguousarray(nb_rep[:, :S_C]),
               np.ascontiguousarray(nb_rep[:, S_C:])]

    # v_k = d2 at own index (exact), fp64 -> fp32 bias tables per query slice
    v = b2_64[idx] - 2.0 * np.einsum(
        "kd,kd->k", a.astype(np.float64), target[idx].astype(np.float64)
    )
    _CACHE["v"] = v
    vba_all = (v - CENTER - DELTA).astype(np.float32)
    vbd_all = (v - DELTA).astype(np.float32)

    at_cores, vba_cores, vbd_cores = [], [], []
    for cq in range(K // NQ):                          # 4 query slices
        sl = slice(cq * NQ, (cq + 1) * NQ)
        at_cores.append(np.ascontiguousarray(
            at_all[:, sl].reshape(DC, P, NQ).transpose(1, 0, 2)
        ).astype(ml_dtypes.float8_e4m3))               # [P, DC, NQ]
        vba_cores.append(np.ascontiguousarray(vba_all[sl].reshape(QB, P).T))
        vbd_cores.append(np.ascontiguousarray(vbd_all[sl].reshape(QB, P).T))

    in_maps = []
    for c in range(NCORES):
        half, cq = c // 4, c % 4
        in_maps.append({
            "at": at_cores[cq], "tt": tt_half[half], "b2r": b2r_half[half],
            "nb": nb_half[half], "vba": vba_cores[cq], "vbd": vbd_cores[cq],
        })
    return in_maps, a, b2_64


def kernel(mapped: np.ndarray, target: np.ndarray, indexes: np.ndarray) -> np.ndarray:
    from concourse.bass_utils import run_bass_kernel_spmd

    mapped = np.asarray(mapped, dtype=np.float32)
    target = np.asarray(target, dtype=np.float32)
    idx = np.asarray(indexes).astype(np.int64)

    in_maps, a, b2_64 = _marshal(mapped, target, idx)

    # ---- run on the 8 NeuronCores (host numpy fallback if the device path
    # fails repeatedly — correctness insurance) ----
    witness = None
    last_exc = None
    for attempt in range(3):
        try:
            nc = _get_nc()
            kwargs = {}
            if os.environ.get("KERNEL_TRACE_DIR"):
                kwargs["tmpdir"] = os.environ["KERNEL_TRACE_DIR"]
            res = run_bass_kernel_spmd(
                nc, in_maps, core_ids=list(range(NCORES)), **kwargs
            )
            _CACHE["last_res"] = res  # exec_time_ns/profile when BASS_TRACE=1
            # accw[p, qb] on core c: measure for query (c%4)*1024 + qb*128 + p
            # over the sampled-target half c//4
            w = np.zeros(K, dtype=np.float64)
            for c in range(NCORES):
                acc = res.results[c]["accw"].astype(np.float64)  # [P, QB]
                cq = c % 4
                w[cq * NQ:(cq + 1) * NQ] += acc.T.reshape(NQ)
            witness = w > 0.0
            break
        except Exception as e:  # noqa: BLE001 - retry/fallback on any device error
            last_exc = e
            _CACHE.pop("nc", None)
    if witness is None:
        sys.stderr.write(f"kernel: device path failed ({last_exc}); host fallback\n")
        witness = np.zeros(K, dtype=bool)

    # ---- host decision: witnessed queries are proven mismatched; the rest
    # get an exact fp64 check ----
    mismatch = witness.copy()
    flagged = np.nonzero(~witness)[0]
    _CACHE["flagged_n"] = len(flagged)
    t64 = None
    for i in range(0, len(flagged), 64):
        blk = flagged[i:i + 64]
        if t64 is None:
            t64 = target.astype(np.float64)
        d2 = b2_64[None, :] - 2.0 * (a[blk].astype(np.float64) @ t64.T)
        mismatch[blk] = np.argmin(d2, axis=1) != idx[blk]

    return np.asarray(mismatch.mean(), dtype=np.float32)


if __name__ == "__main__":
    rng = np.random.default_rng(1)
    mapped = rng.standard_normal((NX, D)).astype(np.float32)
    target = rng.standard_normal((NY, D)).astype(np.float32)
    indexes = rng.integers(0, NY, size=K).astype(np.int32)
    out = kernel(mapped=mapped, target=target, indexes=indexes)
    print("kernel output:", out, out.shape, out.dtype)


# revision 6
# speedup vs baseline: 5.7616x; 1.3682x over previous
"""Trainium2 8-core kernel for nn_AlignedGloveLayer (retrieval 1-NN mismatch loss).

Problem: a = mapped[indexes] ([4096, 256]); d2[k, j] = |a_k - target_j|^2 over
30000 targets; loss = mean over k of (argmin_j d2[k, j] != indexes[k]).

Strategy (witness counting): query k is mismatched iff SOME target j has
d2[k, j] < d2[k, indexes[k]]. The device searches a fixed sampled subset of
S targets for witnesses with margin DELTA (covering all device arithmetic
error): any witness found proves mismatch; queries with no witness are
resolved exactly on the host (a handful for random data, since a query's
own-index distance typically ranks ~uniformly among 30000 distances).

The sampled subset is the S targets whose squared norms b2 are CLOSEST TO THE
MEDIAN b2. Within that band b2_j = B2C +- HW with HW ~2, so b2 folds into the
per-query threshold (widened by HW) and the device never touches b2 at all:
  witness claim:  -2 a_k . t_j < v_k - B2C - (DELTA + HW)
  soundness:      d2_jk = b2_j - 2 a.t < B2C + HW + v_k - B2C - DELTA - HW
                        = v_k - DELTA  (true closer target)

Device layout (queries on PSUM partitions, targets on the free dim):
  2x4 grid: cores 0-3 take 1024 queries each over the first S/2 band targets;
  cores 4-7 the same query slices over the second S/2. Per core, 8 query
  blocks of 128; per block one PSUM tile [128, S_c]:
    psum[q, t] = sum_d T[t, d] * (-2 a[q, d])   (fp8 DoubleRow, 256-deep)
  then ONE fused instruction per tile yields the per-query witness measure:
    ACT: out = Relu(thr_q - psum), accum_out[q] = sum(out)   (>0 iff witness)
    DVE: out = (psum is_lt thr_q), accum_out[q] = count
  Only the [128, 8] accum table is DMA'd out (4KB/core).
"""
import os
import sys

for _p in ("/opt/trn_rl_repo", "/root/.axon_site/_ro/trn_rl_repo"):
    if os.path.isdir(_p) and _p not in sys.path:
        sys.path.append(_p)

from contextlib import ExitStack

import ml_dtypes
import numpy as np

NX, NY, D, K = 30000, 30000, 256, 4096
NCORES = 8
P = 128
DC = D // P          # 2 contraction k-tiles (fp8 DoubleRow: 256-deep)
NQ = 1024            # queries per core (cores c and c+4 share a query slice)
QB = NQ // P         # 8 query blocks
S_TOTAL = 1024       # sampled targets (device witness search set)
S_C = S_TOTAL // 2   # sampled targets per core (two halves)
DELTA = 18.5         # witness margin >= device arithmetic error bound
ACT_SET = (1, 3, 5, 7)  # query blocks routed through ScalarE

_CACHE: dict = {}


def _build_nc():
    import concourse.tile as tile
    from concourse import bacc, mybir
    nc = bacc.Bacc("TRN2", target_bir_lowering=False)
    at_d = nc.dram_tensor("at", [P, DC, NQ], mybir.dt.float8e4, kind="ExternalInput")
    tt_d = nc.dram_tensor("tt", [P, DC, S_C], mybir.dt.float8e4, kind="ExternalInput")
    vb_d = nc.dram_tensor("vb", [P, QB], mybir.dt.float32, kind="ExternalInput")
    accw_d = nc.dram_tensor("accw", [P, QB], mybir.dt.float32, kind="ExternalOutput")

    with tile.TileContext(nc) as tc:
        with ExitStack() as ctx:
            sb = ctx.enter_context(tc.tile_pool(name="sb", bufs=1))
            dump = ctx.enter_context(tc.tile_pool(name="dump", bufs=3))
            nbanks = max(1, S_C // 512)
            psum = ctx.enter_context(
                tc.tile_pool(name="psum", bufs=8 // nbanks, space="PSUM")
            )

            # Inputs spread across the DMA-capable queues (gpsimd/SP only —
            # Scalar stays free for the ACT table load).
            accw = sb.tile([P, QB], mybir.dt.float32)
            nc.gpsimd.memset(accw[:], 0.0)
            warm = sb.tile([P, 1], mybir.dt.float32)
            nc.gpsimd.memset(warm[:], 0.0)
            at = sb.tile([P, DC, NQ], mybir.dt.float8e4)
            nc.gpsimd.dma_start(at[:], at_d[:])
            tt = sb.tile([P, DC, S_C], mybir.dt.float8e4)
            nc.sync.dma_start(tt[:], tt_d[:])
            vb = sb.tile([P, QB], mybir.dt.float32)
            nc.sync.dma_start(vb[:], vb_d[:])
            zz = sb.tile([P, S_C], mybir.dt.bfloat16)
            nc.vector.memset(zz[:], 0.0)

            # Pull the ACT table load off the critical path while DMAs fly.
            nc.scalar.activation(
                warm[:], warm[:], mybir.ActivationFunctionType.Relu,
                bias=0.0, scale=1.0,
            )

            for qb in range(QB):
                ps = psum.tile([P, S_C], mybir.dt.float32)
                for h in range(max(1, S_C // 512)):
                    t0 = h * 512
                    tw = min(512, S_C)
                    nc.tensor.matmul(
                        ps[:, t0:t0 + tw],
                        at[:, :, qb * P:(qb + 1) * P],
                        tt[:, :, t0:t0 + tw],
                        start=True, stop=True,
                        perf_mode=mybir.MatmulPerfMode.DoubleRow,
                    )
                vo = dump.tile([P, S_C], mybir.dt.float16, tag="vo")
                if qb in ACT_SET:
                    # accum[q] = sum_t relu(thr_q - psum) : > 0 iff witness
                    nc.scalar.activation(
                        vo[:], ps[:], mybir.ActivationFunctionType.Relu,
                        bias=vb[:, qb:qb + 1], scale=-1.0,
                        accum_out=accw[:, qb:qb + 1],
                    )
                else:
                    # accum[q] = #targets with (psum - thr_q) < 0
                    nc.vector.scalar_tensor_tensor(
                        vo[:], ps[:], vb[:, qb:qb + 1], zz[:],
                        op0=mybir.AluOpType.subtract,
                        op1=mybir.AluOpType.is_lt,
                        accum_out=accw[:, qb:qb + 1],
                    )

            nc.sync.dma_start(accw_d[:], accw[:])

    nc.compile()
    return nc


def _get_nc():
    if "nc" not in _CACHE:
        _CACHE["nc"] = _build_nc()
    return _CACHE["nc"]


def _marshal(mapped, target, idx):
    """Host-side sharding/quantization. Returns (in_maps, a, b2_64)."""
    a = mapped[idx]                                   # [K, D] fp32
    at_all = np.ascontiguousarray((-2.0 * a).T)       # [D, K]

    b2_64 = (target.astype(np.float64) ** 2).sum(1)   # exact fp64 row norms
    med = np.median(b2_64)
    sidx = np.sort(np.argsort(np.abs(b2_64 - med))[:S_TOTAL])
    _CACHE["sidx"] = sidx
    b2band = b2_64[sidx]
    b2c = float(b2band.mean())
    hw = float(np.abs(b2band - b2c).max())            # band halfwidth
    _CACHE["band"] = (b2c, hw)
    tsub = target[sidx]                               # [S, D]

    # tt[p, dc, t] = tsub[t, dc*128 + p] in fp8
    tt_all = np.ascontiguousarray(
        tsub.reshape(S_TOTAL, DC, P).transpose(2, 1, 0)
    ).astype(ml_dtypes.float8_e4m3)                   # [P, DC, S]
    tt_half = [np.ascontiguousarray(tt_all[:, :, :S_C]),
               np.ascontiguousarray(tt_all[:, :, S_C:])]

    # v_k = d2 at own index (exact); thr = v - b2c - (DELTA + hw)
    v = b2_64[idx] - 2.0 * np.einsum(
        "kd,kd->k", a.astype(np.float64), target[idx].astype(np.float64)
    )
    _CACHE["v"] = v
    thr_all = (v - b2c - (DELTA + hw)).astype(np.float32)

    at_cores, vb_cores = [], []
    for cq in range(K // NQ):                          # 4 query slices
        sl = slice(cq * NQ, (cq + 1) * NQ)
        at_cores.append(np.ascontiguousarray(
            at_all[:, sl].reshape(DC, P, NQ).transpose(1, 0, 2)
        ).astype(ml_dtypes.float8_e4m3))               # [P, DC, NQ]
        vb_cores.append(np.ascontiguousarray(thr_all[sl].reshape(QB, P).T))

    in_maps = []
    for c in range(NCORES):
        half, cq = c // 4, c % 4
        in_maps.append({
            "at": at_cores[cq], "tt": tt_half[half], "vb": vb_cores[cq],
        })
    return in_maps, a, b2_64


def kernel(mapped: np.ndarray, target: np.ndarray, indexes: np.ndarray) -> np.ndarray:
    from concourse.bass_utils import run_bass_kernel_spmd

    mapped = np.asarray(mapped, dtype=np.float32)
    target = np.asarray(target, dtype=np.float32)
    idx = np.asarray(indexes).astype(np.int64)

    in_maps, a, b2_64 = _marshal(mapped, target, idx)

    # ---- run on the 8 NeuronCores (host numpy fallback if the device path
    # fails repeatedly — correctness insurance) ----
    witness = None
    last_exc = None
    for attempt in range(3):
        try:
            nc = _get_nc()
            kwargs = {}
            if os.environ.get("KERNEL_TRACE_DIR"):
                kwargs["tmpdir"] = os.environ["KERNEL_TRACE_DIR"]
            res = run_bass_kernel_spmd(
                nc, in_maps, core_ids=list(range(NCORES)), **kwargs
            )
            _CACHE["last_res"] = res  # exec_time_ns/profile when BASS_TRACE=1
            # accw[p, qb] on core c: measure for query (c%4)*1024 + qb*128 + p
            # over the sampled-target half c//4
            w = np.zeros(K, dtype=np.float64)
            for c in range(NCORES):
                acc = res.results[c]["accw"].astype(np.float64)  # [P, QB]
                cq = c % 4
                w[cq * NQ:(cq + 1) * NQ] += acc.T.reshape(NQ)
            witness = w > 0.0
            break
        except Exception as e:  # noqa: BLE001 - retry/fallback on any device error
            last_exc = e
            _CACHE.pop("nc", None)
    if witness is None:
        sys.stderr.write(f"kernel: device path failed ({last_exc}); host fallback\n")
        witness = np.zeros(K, dtype=bool)

    # ---- host decision: witnessed queries are proven mismatched; the rest
    # get an exact fp64 check ----
    mismatch = witness.copy()
    flagged = np.nonzero(~witness)[0]
    _CACHE["flagged_n"] = len(flagged)
    t64 = None
    for i in range(0, len(flagged), 64):
        blk = flagged[i:i + 64]
        if t64 is None:
            t64 = target.astype(np.float64)
        d2 = b2_64[None, :] - 2.0 * (a[blk].astype(np.float64) @ t64.T)
        mismatch[blk] = np.argmin(d2, axis=1) != idx[blk]

    return np.asarray(mismatch.mean(), dtype=np.float32)


if __name__ == "__main__":
    rng = np.random.default_rng(1)
    mapped = rng.standard_normal((NX, D)).astype(np.float32)
    target = rng.standard_normal((NY, D)).astype(np.float32)
    indexes = rng.integers(0, NY, size=K).astype(np.int32)
    out = kernel(mapped=mapped, target=target, indexes=indexes)
    print("kernel output:", out, out.shape, out.dtype)
